# revision 22
# baseline (speedup 1.0000x reference)
"""CRF forward (log-partition) kernel v4 for Trainium2, 8 NeuronCores.

Chunked linear-recurrence scan in the exp domain (q_t = diag(eu_t) E q_{t-1},
E = exp(transitions), eu = lse-normalized exp(unary)).

Transposed state layout: the state lives as q[j, b] (tags j in partitions,
B=512 chunk-columns in the free dim; L=16 owned steps per chunk). Each step
computes out[i, b] = sum_j ET[j, i] * q[j, b] with the CONSTANT ET slice as
the stationary operand and the state as the moving operand (N=512-wide
streams, fp8 DoubleRow): 32 matmuls of [K=256v, M=128, N=512] per step
filling all 8 PSUM banks; DVE then forms the next state in place:
qnext[:, m, :] = psum[m] * eut[:, m, :] (bf16 -> fp8). No transposes.

The warm-up halo step runs on the HOST (q0 = eu[halo] * rowsum(E), fp8);
the device executes exactly the 8192 owned steps per core as 16 batched
steps. Entering/leaving chunk sums are reduced on host from q0/qfin.

v4 scheduling refinements (from the v3 trace, 133.3us):
- et/q0 live in pair-granular tiles and their DMAs are interleaved ahead of
  the eut prefetches, so step 0's first matmul group gates on 384KB, not on
  the full 4.5MB FIFO (v3 fill was 13.4us).
- m-order [6,7,0..5] with pair g2 accumulated LAST in the first m-block:
  the final psum block's DVE multiply (block 5, ends ~0.7us after the last
  matmul) is consumed at +723ns, eliminating the per-step boundary stall.
- 6 DMA-independent dummy matmuls warm the PE HAM clock gate during the
  initial fill (cold matmuls run at 1.2 GHz for the first ~3.4us).
- last step's qfin export is split per pair in mult-completion order.

Per-step budget: PE 32 MM x 216ns = 6.9us (stream-bound; LDW ~136ns hidden);
DVE 8 x 682ns = 5.5us (trails per-block); DMA 1MB/step sustained 145GB/s.
"""

import numpy as np
import ml_dtypes
from contextlib import ExitStack

T = 65536
N = 1024
NCORES = 8
B = 512           # chunk-columns per core
L = 16            # owned steps per chunk (= device steps)
PERCORE = T // NCORES
PREFETCH = 3      # eut tiles in flight
# dummy matmuls bridging the first-half const fill (~2.6us at the 427ns
# cold rate): they keep the PE busy until et/q0's first halves land (~11us),
# so the HAM clock gate is warming before the first real matmul issues
NWARM = 6

# Matmul schedule: the first three m-blocks (6, 7, 0) interleave round-robin
# over pairs [g0, g1, g3, g2] so the first use of pair g2 sits at position 10
# (+2.16us into the step) — the previous step's final DVE multiply (block 5,
# ready ~+1.1us) comfortably beats it even with semaphore latency. Remaining
# blocks run m-outer; DVE multiplies trail their psum stops with slack.
MM_SCHED = [(m, g) for g in (0, 1, 3, 2) for m in (6, 7, 0)] + [
    (m, g) for m in (1, 2, 3, 4, 5) for g in (0, 1, 2, 3)
]
MULT_ORDER = [6, 7, 0, 1, 2, 3, 4, 5]  # psum-stop completion order
# the last step feeds no successor: plain m-outer staggers the psum stops
# so the final DVE multiply lands ~1.4us after the last matmul (vs ~1.9)
MM_SCHED_LAST = [(m, g) for m in MULT_ORDER for g in (0, 1, 2, 3)]

_BF = ml_dtypes.bfloat16
_F8 = ml_dtypes.float8_e4m3   # TRN FP8_EXP4: max +-240

_compiled = {}


def _build_bass():
    import concourse.bacc as bacc
    import concourse.tile as tile
    from concourse import mybir

    bf = mybir.dt.bfloat16
    f32 = mybir.dt.float32
    f8 = mybir.dt.float8e4
    DR = mybir.MatmulPerfMode.DoubleRow

    nc = bacc.Bacc("TRN2", name="crf_fwd4")

    # eut[128*t + i_in, m, b] = eu'_{owned step t}[chunk b][m*128 + i_in]
    EUT = nc.dram_tensor("eut", [L * 128, 8, B], bf, kind="ExternalInput")
    # et8[k, 2g+ko, i] = E[i, 256g + 128ko + k]
    ET8 = nc.dram_tensor("et8", [128, 8, N], f8, kind="ExternalInput")
    # q0[i_in, m, b] = initial state (host-computed halo step)
    Q0 = nc.dram_tensor("q0", [128, 8, B], f8, kind="ExternalInput")
    QFIN = nc.dram_tensor("qfin", [128, 8, B], f8, kind="ExternalOutput")

    with tile.TileContext(nc) as tc, ExitStack() as ctx:
        consts = ctx.enter_context(tc.tile_pool(name="consts", bufs=1))
        eupool = ctx.enter_context(tc.tile_pool(name="eu", bufs=PREFETCH + 1))
        qpool = ctx.enter_context(tc.tile_pool(name="q", bufs=2))
        pspool = ctx.enter_context(tc.tile_pool(name="ps", bufs=1, space="PSUM"))

        # warm-up operands: no DMA dependency, ready immediately
        warm_w = consts.tile([128, 2, 128], f8)
        warm_x = consts.tile([128, 2, B], f8)
        nc.vector.memset(warm_w[:], 0.0)
        nc.vector.memset(warm_x[:], 0.0)

        # consts as HALF-granular tiles (Tile tracks deps per tile, not per
        # region): the g0/g1 matmuls of step 0 gate on the first 0.75MB
        # (~11us) instead of the full 1.5MB (~13us); pair g3 sits at schedule
        # position 7, meeting the second half right as it lands
        et_h = [consts.tile([128, 4, N], f8, name=f"et_h{h}") for h in range(2)]
        q0_h = [consts.tile([128, 4, B], f8, name=f"q0_h{h}") for h in range(2)]
        for h in range(2):
            nc.sync.dma_start(out=et_h[h][:], in_=ET8[:, 4 * h : 4 * h + 4, :])
            nc.sync.dma_start(out=q0_h[h][:], in_=Q0[:, 4 * h : 4 * h + 4, :])

        def load_eut(t):
            # all eut tiles ride the Sync ring BEHIND the consts: strict
            # FIFO priority. (Putting any early tile on the ACT ring makes
            # its data race the consts on the shared SDMA engines, stalling
            # steps 1-3 long enough to re-throttle the PE clock gate.)
            til = eupool.tile([128, 8, B], bf, tag="eu", name=f"eu_{t}")
            if t == 0:
                # m-slice chunks in consumption order: the step-0 DVE
                # multiplies gate on 128KB each instead of the full 1MB
                for mlo in (6, 0, 2, 4):
                    nc.sync.dma_start(
                        out=til[:, mlo : mlo + 2, :],
                        in_=EUT[0:128, mlo : mlo + 2, :],
                    )
            else:
                nc.sync.dma_start(out=til[:], in_=EUT[128 * t : 128 * (t + 1), :, :])
            return til

        eu_tiles = [load_eut(t) for t in range(PREFETCH)]

        # HAM warm-up: keep PE busy while the const DMAs land
        warm_ps = pspool.tile([128, B], f32, tag=f"ps{MULT_ORDER[0]}", name="warm_ps")
        for w in range(NWARM):
            nc.tensor.matmul(
                warm_ps[:], warm_w[:], warm_x[:],
                start=True, stop=True, perf_mode=DR,
            )

        qn = None
        for t in range(L):
            eu_t = eu_tiles[t]
            if t + PREFETCH < L:
                eu_tiles.append(load_eut(t + PREFETCH))
            qprev = qn
            qn = qpool.tile([128, 8, B], f8, tag="qn", name=f"q_{t}")
            ps = {
                m: pspool.tile([128, B], f32, tag=f"ps{m}", name=f"ps{m}_{t}")
                for m in range(8)
            }
            nseen = {m: 0 for m in range(8)}
            for m, g in (MM_SCHED_LAST if t == L - 1 else MM_SCHED):
                h, gg = g // 2, g % 2
                rhs = (
                    q0_h[h][:, 2 * gg : 2 * gg + 2, :]
                    if t == 0
                    else qprev[:, 2 * g : 2 * g + 2, :]
                )
                nc.tensor.matmul(
                    ps[m][:],
                    et_h[h][:, 2 * gg : 2 * gg + 2, 128 * m : 128 * (m + 1)],
                    rhs,
                    start=(nseen[m] == 0),
                    stop=(nseen[m] == 3),
                    perf_mode=DR,
                )
                nseen[m] += 1
                if nseen[m] == 4:
                    # qnext[i, b] = psum * eu' (f32 x bf16 -> fp8, 1 rounding)
                    nc.vector.tensor_mul(qn[:, m, :], ps[m][:], eu_t[:, m, :])
                    if t == L - 1 and m in (7, 1, 3, 5):
                        # pair-grouped exports in multiply-completion order;
                        # early pairs' DMAs fully overlap the remaining step
                        g_exp = m // 2
                        nc.sync.dma_start(
                            out=QFIN[:, 2 * g_exp : 2 * g_exp + 2, :],
                            in_=qn[:, 2 * g_exp : 2 * g_exp + 2, :],
                        )

    nc.finalize()
    return nc


def _get_nc():
    if "nc" not in _compiled:
        _compiled["nc"] = _build_bass()
    return _compiled["nc"]


def _prep_inputs(unary, transitions, start_idx):
    """Host-side: lse-normalized exp(unary) in bf16 (transposed layout),
    fp8 E^T, and the fp8 initial state q0 (halo step done on host)."""
    unary = np.asarray(unary, dtype=np.float32)
    transitions = np.asarray(transitions, dtype=np.float32)

    m = unary.max(axis=1)
    lse = m + np.log(np.exp(unary - m[:, None]).sum(axis=1, dtype=np.float32))
    _compiled["lse_sum"] = float(lse.astype(np.float64).sum())
    eu = np.exp(unary - lse[:, None])  # f32, rows sum to 1

    E = np.exp(transitions)  # [i, j], entries in ~[0.6, 1.7]
    # et8[k, 2g+ko, i] = E[i, 256g+128ko+k]
    et8 = np.ascontiguousarray(
        E.T.reshape(4, 2, 128, N).transpose(2, 0, 1, 3).reshape(128, 8, N)
    ).astype(_F8)
    rowsum = E.sum(axis=1)  # [i]

    in_maps = []
    s0_all = []
    for c in range(NCORES):
        ec = eu[c * PERCORE : (c + 1) * PERCORE]
        # eut[t, i_in, mm, b] = ec[16b + t, 128mm + i_in]
        eut = np.ascontiguousarray(
            ec.reshape(B, L, N)
            .transpose(1, 2, 0)
            .reshape(L, 8, 128, B)
            .transpose(0, 2, 1, 3)
        ).astype(_BF)

        # halo rows: previous row before each chunk's first owned row
        h_idx = c * PERCORE + L * np.arange(B) - 1
        if c == 0:
            h_idx[0] = 0  # dummy, overwritten below
        q0 = eu[h_idx] * rowsum[None, :] * 0.25  # [b, i]
        if c == 0:
            # one-hot start; scale 128 keeps step-1 entries (~128*eu*E)
            # clear of the fp8 subnormal flush threshold
            q0[0] = 0.0
            q0[0, start_idx] = 128.0
        q0 = np.clip(q0, 0.0, 240.0).astype(_F8)
        q0t = np.ascontiguousarray(
            q0.T.reshape(8, 128, B).transpose(1, 0, 2)
        )  # [i_in, m, b]
        s0_all.append(q0t.astype(np.float32).sum(axis=(0, 1)))  # [b]

        in_maps.append(
            {"eut": eut.reshape(L * 128, 8, B), "et8": et8, "q0": q0t}
        )
    _compiled["s0"] = np.concatenate(s0_all)
    return in_maps


def _combine(results, transitions, end_idx):
    transitions = np.asarray(transitions, dtype=np.float32)
    s0 = _compiled["s0"].astype(np.float64)
    s1 = np.concatenate(
        [r["qfin"].astype(np.float32).sum(axis=(0, 1)) for r in results]
    ).astype(np.float64)
    total = _compiled["lse_sum"] + float(np.sum(np.log(s1) - np.log(s0)))
    # final state of the last chunk: q_T[m*128 + i_in] = qfin[i_in, m, B-1]
    q_T = results[-1]["qfin"][:, :, B - 1].T.reshape(-1).astype(np.float64)
    tau = np.exp(transitions[end_idx].astype(np.float64))
    total += float(np.log(np.dot(tau, q_T))) - float(np.log(s1[-1]))
    return total


def kernel(unary, transitions, start_idx, end_idx, _trace=False):
    from concourse.bass_utils import run_bass_kernel_spmd

    start_idx = int(np.asarray(start_idx))
    end_idx = int(np.asarray(end_idx))

    nc = _get_nc()
    in_maps = _prep_inputs(unary, transitions, start_idx)
    res = run_bass_kernel_spmd(nc, in_maps, core_ids=list(range(NCORES)), trace=_trace)
    _compiled["last_result"] = res
    logZ = _combine(res.results, transitions, end_idx)
    return np.array(logZ, dtype=np.float32)


# revision 24
# speedup vs baseline: 1.0166x; 1.0166x over previous
"""CRF forward (log-partition) kernel v4 for Trainium2, 8 NeuronCores.

Chunked linear-recurrence scan in the exp domain (q_t = diag(eu_t) E q_{t-1},
E = exp(transitions), eu = lse-normalized exp(unary)).

Transposed state layout: the state lives as q[j, b] (tags j in partitions,
B=512 chunk-columns in the free dim; L=16 owned steps per chunk). Each step
computes out[i, b] = sum_j ET[j, i] * q[j, b] with the CONSTANT ET slice as
the stationary operand and the state as the moving operand (N=512-wide
streams, fp8 DoubleRow): 32 matmuls of [K=256v, M=128, N=512] per step
filling all 8 PSUM banks; DVE then forms the next state in place:
qnext[:, m, :] = psum[m] * eut[:, m, :] (bf16 -> fp8). No transposes.

The warm-up halo step runs on the HOST (q0 = eu[halo] * rowsum(E), fp8);
the device executes exactly the 8192 owned steps per core as 16 batched
steps. Entering/leaving chunk sums are reduced on host from q0/qfin.

v4 scheduling refinements (from the v3 trace, 133.3us):
- et/q0 live in pair-granular tiles and their DMAs are interleaved ahead of
  the eut prefetches, so step 0's first matmul group gates on 384KB, not on
  the full 4.5MB FIFO (v3 fill was 13.4us).
- m-order [6,7,0..5] with pair g2 accumulated LAST in the first m-block:
  the final psum block's DVE multiply (block 5, ends ~0.7us after the last
  matmul) is consumed at +723ns, eliminating the per-step boundary stall.
- 6 DMA-independent dummy matmuls warm the PE HAM clock gate during the
  initial fill (cold matmuls run at 1.2 GHz for the first ~3.4us).
- last step's qfin export is split per pair in mult-completion order.

Per-step budget: PE 32 MM x 216ns = 6.9us (stream-bound; LDW ~136ns hidden);
DVE 8 x 682ns = 5.5us (trails per-block); DMA 1MB/step sustained 145GB/s.
"""

import numpy as np
import ml_dtypes
from contextlib import ExitStack

T = 65536
N = 1024
NCORES = 8
B = 512           # chunk-columns per core
L = 16            # owned steps per chunk (= device steps)
PERCORE = T // NCORES
PREFETCH = 3      # eut tiles in flight
# dummy matmuls bridging the whole const fill (~4us at the 427ns cold rate):
# they keep the PE busy until the consts land at ~12.2us, so the HAM clock
# gate flips to 2.4GHz before the first real matmul issues (9 measured best:
# 6 leaves a 1.9us idle gap that re-throttles the clock gate; 11 overshoots)
NWARM = 10

# Matmul schedule: the first three m-blocks (6, 7, 0) interleave round-robin
# over pairs [g0, g1, g3, g2] so the first use of pair g2 sits at position 10
# (+2.16us into the step) — the previous step's final DVE multiply (block 5,
# ready ~+1.1us) comfortably beats it even with semaphore latency. Remaining
# blocks run m-outer; DVE multiplies trail their psum stops with slack.
MM_SCHED = [(m, g) for g in (0, 1, 3, 2) for m in (6, 7, 0)] + [
    (m, g) for m in (1, 2, 3, 4, 5) for g in (0, 1, 2, 3)
]
MULT_ORDER = [6, 7, 0, 1, 2, 3, 4, 5]  # psum-stop completion order
# the last step feeds no successor: plain m-outer staggers the psum stops
# so the final DVE multiply lands ~1.4us after the last matmul (vs ~1.9)
MM_SCHED_LAST = [(m, g) for m in MULT_ORDER for g in (0, 1, 2, 3)]

_BF = ml_dtypes.bfloat16
_F8 = ml_dtypes.float8_e4m3   # TRN FP8_EXP4: max +-240

_compiled = {}


def _build_bass():
    import concourse.bacc as bacc
    import concourse.tile as tile
    from concourse import mybir

    bf = mybir.dt.bfloat16
    f32 = mybir.dt.float32
    f8 = mybir.dt.float8e4
    DR = mybir.MatmulPerfMode.DoubleRow

    nc = bacc.Bacc("TRN2", name="crf_fwd4")

    # eut[128*t + i_in, m, b] = eu'_{owned step t}[chunk b][m*128 + i_in]
    EUT = nc.dram_tensor("eut", [L * 128, 8, B], bf, kind="ExternalInput")
    # et8[k, 2g+ko, i] = E[i, 256g + 128ko + k]
    ET8 = nc.dram_tensor("et8", [128, 8, N], f8, kind="ExternalInput")
    # q0[i_in, m, b] = initial state (host-computed halo step)
    Q0 = nc.dram_tensor("q0", [128, 8, B], f8, kind="ExternalInput")
    QFIN = nc.dram_tensor("qfin", [128, 8, B], f8, kind="ExternalOutput")

    with tile.TileContext(nc) as tc, ExitStack() as ctx:
        consts = ctx.enter_context(tc.tile_pool(name="consts", bufs=1))
        eupool = ctx.enter_context(tc.tile_pool(name="eu", bufs=PREFETCH + 1))
        qpool = ctx.enter_context(tc.tile_pool(name="q", bufs=2))
        pspool = ctx.enter_context(tc.tile_pool(name="ps", bufs=1, space="PSUM"))

        # warm-up operands: no DMA dependency, ready immediately
        warm_w = consts.tile([128, 2, 128], f8)
        warm_x = consts.tile([128, 2, B], f8)
        nc.vector.memset(warm_w[:], 0.0)
        nc.vector.memset(warm_x[:], 0.0)

        # consts in 4 half-tensor DMAs: fewer ~600ns issue slots on the Sync
        # ring means the last const byte lands ~2us earlier than 8 chunks
        et_sb = consts.tile([128, 8, N], f8)
        q0_sb = consts.tile([128, 8, B], f8)
        for lo in (0, 4):
            nc.sync.dma_start(out=et_sb[:, lo : lo + 4, :], in_=ET8[:, lo : lo + 4, :])
            nc.sync.dma_start(out=q0_sb[:, lo : lo + 4, :], in_=Q0[:, lo : lo + 4, :])

        def load_eut(t):
            # all eut tiles ride the Sync ring BEHIND the consts: strict
            # FIFO priority. (Putting any early tile on the ACT ring makes
            # its data race the consts on the shared SDMA engines, stalling
            # steps 1-3 long enough to re-throttle the PE clock gate.)
            til = eupool.tile([128, 8, B], bf, tag="eu", name=f"eu_{t}")
            if t == 0:
                # m-slice chunks in consumption order: the step-0 DVE
                # multiplies gate on 128KB each instead of the full 1MB
                for mlo in (6, 0, 2, 4):
                    nc.sync.dma_start(
                        out=til[:, mlo : mlo + 2, :],
                        in_=EUT[0:128, mlo : mlo + 2, :],
                    )
            else:
                nc.sync.dma_start(out=til[:], in_=EUT[128 * t : 128 * (t + 1), :, :])
            return til

        eu_tiles = [load_eut(t) for t in range(PREFETCH)]

        # HAM warm-up: keep PE busy while the const DMAs land
        warm_ps = pspool.tile([128, B], f32, tag=f"ps{MULT_ORDER[0]}", name="warm_ps")
        for w in range(NWARM):
            nc.tensor.matmul(
                warm_ps[:], warm_w[:], warm_x[:],
                start=True, stop=True, perf_mode=DR,
            )

        qn = None
        for t in range(L):
            eu_t = eu_tiles[t]
            if t + PREFETCH < L:
                eu_tiles.append(load_eut(t + PREFETCH))
            qprev = qn
            qn = qpool.tile([128, 8, B], f8, tag="qn", name=f"q_{t}")
            ps = {
                m: pspool.tile([128, B], f32, tag=f"ps{m}", name=f"ps{m}_{t}")
                for m in range(8)
            }
            nseen = {m: 0 for m in range(8)}
            src = q0_sb if t == 0 else qprev
            for m, g in (MM_SCHED_LAST if t == L - 1 else MM_SCHED):
                nc.tensor.matmul(
                    ps[m][:],
                    et_sb[:, 2 * g : 2 * g + 2, 128 * m : 128 * (m + 1)],
                    src[:, 2 * g : 2 * g + 2, :],
                    start=(nseen[m] == 0),
                    stop=(nseen[m] == 3),
                    perf_mode=DR,
                )
                nseen[m] += 1
                if nseen[m] == 4:
                    # qnext[i, b] = psum * eu' (f32 x bf16 -> fp8, 1 rounding)
                    nc.vector.tensor_mul(qn[:, m, :], ps[m][:], eu_t[:, m, :])
                    if t == L - 1 and m in (7, 1, 3, 5):
                        # pair-grouped exports in multiply-completion order;
                        # early pairs' DMAs fully overlap the remaining step
                        g_exp = m // 2
                        nc.sync.dma_start(
                            out=QFIN[:, 2 * g_exp : 2 * g_exp + 2, :],
                            in_=qn[:, 2 * g_exp : 2 * g_exp + 2, :],
                        )

    nc.finalize()
    return nc


def _get_nc():
    if "nc" not in _compiled:
        _compiled["nc"] = _build_bass()
    return _compiled["nc"]


def _prep_inputs(unary, transitions, start_idx):
    """Host-side: lse-normalized exp(unary) in bf16 (transposed layout),
    fp8 E^T, and the fp8 initial state q0 (halo step done on host)."""
    unary = np.asarray(unary, dtype=np.float32)
    transitions = np.asarray(transitions, dtype=np.float32)

    m = unary.max(axis=1)
    lse = m + np.log(np.exp(unary - m[:, None]).sum(axis=1, dtype=np.float32))
    _compiled["lse_sum"] = float(lse.astype(np.float64).sum())
    eu = np.exp(unary - lse[:, None])  # f32, rows sum to 1

    E = np.exp(transitions)  # [i, j], entries in ~[0.6, 1.7]
    # et8[k, 2g+ko, i] = E[i, 256g+128ko+k]
    et8 = np.ascontiguousarray(
        E.T.reshape(4, 2, 128, N).transpose(2, 0, 1, 3).reshape(128, 8, N)
    ).astype(_F8)
    rowsum = E.sum(axis=1)  # [i]

    in_maps = []
    s0_all = []
    for c in range(NCORES):
        ec = eu[c * PERCORE : (c + 1) * PERCORE]
        # eut[t, i_in, mm, b] = ec[16b + t, 128mm + i_in]
        eut = np.ascontiguousarray(
            ec.reshape(B, L, N)
            .transpose(1, 2, 0)
            .reshape(L, 8, 128, B)
            .transpose(0, 2, 1, 3)
        ).astype(_BF)

        # halo rows: previous row before each chunk's first owned row
        h_idx = c * PERCORE + L * np.arange(B) - 1
        if c == 0:
            h_idx[0] = 0  # dummy, overwritten below
        q0 = eu[h_idx] * rowsum[None, :] * 0.25  # [b, i]
        if c == 0:
            # one-hot start; scale 128 keeps step-1 entries (~128*eu*E)
            # clear of the fp8 subnormal flush threshold
            q0[0] = 0.0
            q0[0, start_idx] = 128.0
        q0 = np.clip(q0, 0.0, 240.0).astype(_F8)
        q0t = np.ascontiguousarray(
            q0.T.reshape(8, 128, B).transpose(1, 0, 2)
        )  # [i_in, m, b]
        s0_all.append(q0t.astype(np.float32).sum(axis=(0, 1)))  # [b]

        in_maps.append(
            {"eut": eut.reshape(L * 128, 8, B), "et8": et8, "q0": q0t}
        )
    _compiled["s0"] = np.concatenate(s0_all)
    return in_maps


def _combine(results, transitions, end_idx):
    transitions = np.asarray(transitions, dtype=np.float32)
    s0 = _compiled["s0"].astype(np.float64)
    s1 = np.concatenate(
        [r["qfin"].astype(np.float32).sum(axis=(0, 1)) for r in results]
    ).astype(np.float64)
    total = _compiled["lse_sum"] + float(np.sum(np.log(s1) - np.log(s0)))
    # final state of the last chunk: q_T[m*128 + i_in] = qfin[i_in, m, B-1]
    q_T = results[-1]["qfin"][:, :, B - 1].T.reshape(-1).astype(np.float64)
    tau = np.exp(transitions[end_idx].astype(np.float64))
    total += float(np.log(np.dot(tau, q_T))) - float(np.log(s1[-1]))
    return total


def kernel(unary, transitions, start_idx, end_idx, _trace=False):
    from concourse.bass_utils import run_bass_kernel_spmd

    start_idx = int(np.asarray(start_idx))
    end_idx = int(np.asarray(end_idx))

    nc = _get_nc()
    in_maps = _prep_inputs(unary, transitions, start_idx)
    res = run_bass_kernel_spmd(nc, in_maps, core_ids=list(range(NCORES)), trace=_trace)
    _compiled["last_result"] = res
    logZ = _combine(res.results, transitions, end_idx)
    return np.array(logZ, dtype=np.float32)
